# revision 1
# baseline (speedup 1.0000x reference)
"""Additive (Bahdanau) attention log-softmax weights on 8 TRN2 NeuronCores.

Math (per batch b, head 0):
    qp = Q @ Wq^T ; kp = K @ Wk^T          (Wc = [Wq | Wk], both [D, D])
    logit[q, k] = Wl . tanh(qp[q] + kp[k] + bc) + bl + where(mask[k]==0, -1e9, 1.0)
    out[q, :]   = log_softmax(logit[q, :])

Distribution: pure data parallel, core c <- (batch b = c//2, q-half c%2),
no collectives.  Sparse-attention trick: keys with mask==0 only need
out = -1e9 - LSE (error O(1) vs magnitude 1e9), so the device only computes
tanh over the ~136 valid keys (host compacts + pads to V).

Device layout per core (q = 128 local rows, V padded valid keys, e = D = 512):
  - PE: qp^T/kp^T projections ([e,q],[e,kc]) with d on partitions.
  - DVE: arg[e, (q,kc)] = kp^T[e,kc] + (qp^T+bc)[e,q] via per-q tensor_scalar
    (bf16 in/out -> 4x mode).
  - ACT: tanh on [128, G*V] tiles (big free dim amortizes instr overhead).
  - PE: Wl-reduce per q-pair (N=2V) into PSUM partition bases {0,64}; the
    stationary is [128,64] with Wl in col 0 / zeros elsewhere so every PSUM
    partition is written (avoids uninitialized-read races).
  - copy PSUM->SBUF staging (engine-any), one gpsimd-queue DMA row-gather per
    4 tiles into a dense [64, 2V] logits tile, then log-softmax (exp is safe
    without max subtraction: |logit| <= sum|Wl| + 1 < 9).
"""

import numpy as np
import ml_dtypes
from contextlib import ExitStack

import concourse.bass as bass
import concourse.tile as tile
from concourse import bacc, mybir
from concourse.bass_utils import run_bass_kernel_spmd

F32 = mybir.dt.float32
BF16 = mybir.dt.bfloat16
AF = mybir.ActivationFunctionType

B, H, Lq, Lkv, D = 4, 1, 256, 256, 512
NCORES = 8
LQL = Lq // 2          # q rows per core
G = 32                 # q rows per tanh tile
NEG = -1.0e9

_nc_cache: dict[int, object] = {}


def _build(V: int, repeats: int = 1):
    """Build + schedule the per-core Bass graph for padded-valid-count V.

    repeats > 1 emits the full body that many times (for slope timing);
    the real kernel uses repeats=1 and the single "out" parameter."""
    W = 2 * V
    nc = bacc.Bacc(None, target_bir_lowering=False)

    p_qt = nc.declare_dram_parameter("qt", [128, 512], BF16, isOutput=False)
    p_kt = nc.declare_dram_parameter("kt", [128, 4 * V], BF16, isOutput=False)
    p_wct = nc.declare_dram_parameter("wct", [4, 128, 1024], BF16, isOutput=False)
    p_bcp = nc.declare_dram_parameter("bcp", [128, 4], F32, isOutput=False)
    p_wlg = nc.declare_dram_parameter("wlg", [128, 256], BF16, isOutput=False)
    p_bv = nc.declare_dram_parameter("bv", [64, W], F32, isOutput=False)
    p_outs = [nc.declare_dram_parameter("out" if r == 0 else f"out{r}",
                                        [64, W + 2], F32, isOutput=True)
              for r in range(repeats)]

    with ExitStack() as ctx:
        tc = ctx.enter_context(tile.TileContext(nc))
        const = ctx.enter_context(tc.tile_pool(name="const", bufs=1))
        apool = ctx.enter_context(tc.tile_pool(name="apool", bufs=6))
        tpool = ctx.enter_context(tc.tile_pool(name="tpool", bufs=9))
        spool = ctx.enter_context(tc.tile_pool(name="spool", bufs=3))
        psum = ctx.enter_context(tc.tile_pool(name="psum", bufs=8, space="PSUM"))

        for rep in range(repeats):
            # ---- loads (few big DMAs: each dma_start costs ~0.5us of queue
            #      dispatch, so tiles are packed column-wise into single tensors) --
            wct_t = []
            for ec in range(4):   # col = half*512 + dc*128 + e'
                t = const.tile([128, 1024], BF16, tag=f"wct{ec}", name=f"wct{ec}_r{rep}")
                wct_t.append(t)
            nc.sync.dma_start(wct_t[0][:], p_wct[0])
            bcp_t = const.tile([128, 4], F32, tag="bcp", name=f"bcp_r{rep}")
            nc.sync.dma_start(bcp_t[:], p_bcp[:])
            kt_t = const.tile([128, 4 * V], BF16, tag="kt", name=f"kt_r{rep}")   # col = dc*V + kc
            nc.sync.dma_start(kt_t[:], p_kt[:])
            qt_t = const.tile([128, 512], BF16, tag="qt", name=f"qt_r{rep}")     # col = dc*128 + q
            nc.sync.dma_start(qt_t[:], p_qt[:])
            for ec in range(1, 4):
                nc.sync.dma_start(wct_t[ec][:], p_wct[ec])
            wlg_t = const.tile([128, 256], BF16, tag="wlg", name=f"wlg_r{rep}")
            nc.sync.dma_start(wlg_t[:], p_wlg[:])
            bv_t = const.tile([64, W], F32, tag="bv", name=f"bv_r{rep}")
            nc.sync.dma_start(bv_t[:], p_bv[:])

            # ---- phase 2: tanh + Wl-reduce (phase-1 projections are emitted
            #      inside the first group, per ec, so the DVE stream interleaves
            #      them with the first arg-adds instead of blocking on wct3) ----
            qpbc = const.tile([128, D], F32, tag="qpbc", name=f"qpbc_r{rep}")     # col = ec*128 + q
            kpb = const.tile([128, 4 * V], BF16, tag="kpb", name=f"kpb_r{rep}")  # col = ec*V + kc
            lg = const.tile([64, W], F32, tag="lg", name=f"lg_r{rep}")  # row = q-pair, col = (q%2)*V+kc
            lgb = const.tile([64, W], F32, tag="lgb", name=f"lgb_r{rep}")
            ex = const.tile([64, W], F32, tag="ex", name=f"ex_r{rep}")
            sm = const.tile([64, 2], F32, tag="sm", name=f"sm_r{rep}")
            lsm = const.tile([64, 2], F32, tag="lsm", name=f"lsm_r{rep}")
            outv = const.tile([64, W], F32, tag="outv", name=f"outv_r{rep}")
            fill = const.tile([64, 2], F32, tag="fill", name=f"fill_r{rep}")
            # group sizes: small first group -> earlier ACT start; small last
            # group -> shorter serial tail
            GS = [16, 32, 32, 32, 16]
            qbase = 0
            sm_done = 0
            for qg, Gg in enumerate(GS):
                thts = []
                for ec in range(4):
                    if qg == 0:
                        ps2 = psum.tile([128, V], F32, tag="ps", name=f"psk{ec}_r{rep}")
                        for dc in range(4):
                            nc.tensor.matmul(
                                ps2[:],
                                wct_t[ec][:, 512 + dc * 128:512 + (dc + 1) * 128],
                                kt_t[:, dc * V:(dc + 1) * V],
                                start=(dc == 0), stop=(dc == 3))
                        nc.vector.tensor_copy(kpb[:, ec * V:(ec + 1) * V], ps2[:])
                        ps = psum.tile([128, 128], F32, tag="ps", name=f"psq{ec}_r{rep}")
                        for dc in range(4):
                            nc.tensor.matmul(
                                ps[:], wct_t[ec][:, dc * 128:(dc + 1) * 128],
                                qt_t[:, dc * 128:(dc + 1) * 128],
                                start=(dc == 0), stop=(dc == 3))
                        nc.vector.tensor_scalar_add(
                            qpbc[:, ec * 128:(ec + 1) * 128], ps[:],
                            bcp_t[:, ec:ec + 1])
                    a = apool.tile([128, Gg * V], BF16, tag="arg", name=f"arg{qg}_{ec}_r{rep}")
                    for j in range(Gg):
                        q = qbase + j
                        nc.vector.tensor_scalar_add(
                            a[:, j * V:(j + 1) * V],
                            kpb[:, ec * V:(ec + 1) * V],
                            qpbc[:, ec * 128 + q: ec * 128 + q + 1])
                    t = tpool.tile([128, Gg * V], BF16, tag="tht",
                                   name=f"tht{qg}_{ec}_r{rep}")
                    nc.scalar.activation(t[:], a[:], AF.Tanh)
                    thts.append(t)
                ntile = Gg // 4
                ptiles = [psum.tile([128, W], F32, tag="ps", name=f"pt{qg}_{i}_r{rep}")
                          for i in range(ntile)]
                # ec-outer keeps the PE stationary constant across a run of
                # matmuls (one ldweights per run on hardware)
                for ec, p_loc in [(ec, p) for ec in range(4)
                                  for p in range(Gg // 2)]:
                    tl, r = p_loc // 2, p_loc % 2
                    dst = ptiles[tl][64 * r: 64 * r + 64, :]
                    nc.tensor.matmul(
                        dst, wlg_t[:, ec * 64:ec * 64 + 64],
                        thts[ec][:, (2 * p_loc) * V:(2 * p_loc + 2) * V],
                        start=(ec == 0), stop=(ec == 3),
                        # the sim's zero-region tracker ignores the partition
                        # base, so the two disjoint 64-row groups per bank
                        # falsely collide; per-partition accumulation state is
                        # independent on HW
                        skip_group_check=True)
                for sg in range(ntile // 4):
                    st = spool.tile([128, 4 * W], F32, tag="st",
                                    name=f"st{qg}_{sg}_r{rep}")
                    for c in range(4):
                        nc.any.tensor_copy(st[:, c * W:(c + 1) * W],
                                           ptiles[sg * 4 + c][:])
                    # row r of the stage tile holds 4 pair-rows (one per column
                    # chunk) -> lg rows are filled in block order; the host
                    # unshard applies the matching permutation
                    for r in range(2):
                        base = qbase // 2 + sg * 8 + r * 4
                        nc.gpsimd.dma_start(
                            lg[base: base + 4, :], st[64 * r:64 * r + 1, :])
                qbase += Gg
                # softmax front per 32-row half (engine ops need 32-aligned
                # partition starts); exp's accum_out fuses the pair-sums
                while sm_done < qbase // 2 - 31:
                    r0 = sm_done
                    r1 = r0 + 32
                    nc.vector.tensor_add(lgb[r0:r1, :], lg[r0:r1, :], bv_t[r0:r1, :])
                    nc.scalar.activation(ex[r0:r1, 0:V], lgb[r0:r1, 0:V], AF.Exp,
                                         accum_out=sm[r0:r1, 0:1])
                    nc.scalar.activation(ex[r0:r1, V:W], lgb[r0:r1, V:W], AF.Exp,
                                         accum_out=sm[r0:r1, 1:2])
                    sm_done = r1

            # ---- softmax tail: Ln grouped once (one act-table switch) ----
            nc.scalar.activation(lsm[:], sm[:], AF.Ln)
            nc.vector.tensor_scalar_sub(outv[:, 0:V], lgb[:, 0:V], lsm[:, 0:1])
            nc.vector.tensor_scalar_sub(outv[:, V:W], lgb[:, V:W], lsm[:, 1:2])
            nc.vector.tensor_scalar(
                fill[:], lsm[:], -1.0, NEG,
                op0=mybir.AluOpType.mult, op1=mybir.AluOpType.add)
            nc.sync.dma_start(p_outs[rep][:, 0:W], outv[:])
            nc.scalar.dma_start(p_outs[rep][:, W:W + 2], fill[:])

    nc.compile()
    return nc


def _prep(queries, keys, values, mask, Wc, bc, Wl, bl):
    """Host-side sharding: returns (V, in_maps, idx_valid, idx_masked)."""
    mask = np.asarray(mask)
    idx_v = [np.nonzero(mask[b])[0] for b in range(B)]
    idx_m = [np.nonzero(mask[b] == 0)[0] for b in range(B)]
    maxv = max(len(ix) for ix in idx_v)
    V = max(136, -(-maxv // 8) * 8)

    bf = ml_dtypes.bfloat16
    wct_full = np.asarray(Wc, np.float32).T.astype(bf)       # [2D, D]
    wct = np.empty((4, 128, 1024), bf)
    for ec in range(4):
        for dc in range(4):
            wct[ec, :, dc * 128:(dc + 1) * 128] = \
                wct_full[dc * 128:(dc + 1) * 128, ec * 128:(ec + 1) * 128]
            wct[ec, :, 512 + dc * 128:512 + (dc + 1) * 128] = \
                wct_full[D + dc * 128:D + (dc + 1) * 128, ec * 128:(ec + 1) * 128]
    bcp = np.ascontiguousarray(np.asarray(bc, np.float32).reshape(4, 128).T)
    wlg = np.zeros((128, 256), bf)
    wlg[:, 0::64] = np.asarray(Wl, np.float32)[0].reshape(4, 128).T.astype(bf)
    blv = float(np.asarray(bl, np.float32)[0])

    q_np = np.asarray(queries, np.float32)
    k_np = np.asarray(keys, np.float32)
    in_maps = []
    for c in range(NCORES):
        b, qh = c // 2, c % 2
        qt_d = q_np[b, 0, qh * LQL:(qh + 1) * LQL, :].T.astype(bf)   # [D, LQL]
        qt = np.ascontiguousarray(
            qt_d.reshape(4, 128, LQL).transpose(1, 0, 2).reshape(128, 512))
        ktc = np.zeros((D, V), bf)
        ktc[:, :len(idx_v[b])] = k_np[b, 0, idx_v[b], :].T.astype(bf)
        kt = np.ascontiguousarray(
            ktc.reshape(4, 128, V).transpose(1, 0, 2).reshape(128, 4 * V))
        bvrow = np.full(V, NEG, np.float32)
        bvrow[:len(idx_v[b])] = 1.0 + blv
        bv = np.tile(np.concatenate([bvrow, bvrow]), (64, 1))
        in_maps.append({
            "qt": qt, "kt": kt, "wct": wct,
            "bcp": bcp, "wlg": wlg, "bv": np.ascontiguousarray(bv),
        })
    return V, in_maps, idx_v, idx_m


def _unshard_rows(o, V):
    """[64, 2V+2] device rows -> ([pair, q%2, kc] values, [pair, q%2] fill).

    lg row (qg, sg, r, c) holds q-pair qg*16 + sg*8 + 2*c + r."""
    W = 2 * V
    perm = np.empty(64, np.int64)
    for qg in range(4):
        for sg in range(2):
            for r in range(2):
                for cc in range(4):
                    perm[qg * 16 + sg * 8 + 2 * cc + r] = \
                        qg * 16 + sg * 8 + r * 4 + cc
    return o[perm, :W].reshape(64, 2, V), o[perm, W:W + 2]


def kernel(queries, keys, values, mask, Wc, bc, Wl, bl):
    V, in_maps, idx_v, idx_m = _prep(queries, keys, values, mask, Wc, bc, Wl, bl)
    if V not in _nc_cache:
        _nc_cache[V] = _build(V)
    nc = _nc_cache[V]
    res = run_bass_kernel_spmd(nc, in_maps, core_ids=list(range(NCORES))).results

    full = np.empty((B, Lq, Lkv), np.float32)
    for c in range(NCORES):
        b, qh = c // 2, c % 2
        vals, fl = _unshard_rows(np.asarray(res[c]["out"], np.float32), V)
        nv = len(idx_v[b])
        blk = full[b, qh * LQL:(qh + 1) * LQL]          # [128, Lkv]
        blk[:, idx_v[b]] = vals[:, :, :nv].reshape(LQL, nv)
        blk[:, idx_m[b]] = fl.reshape(LQL, 1)
    return full



# revision 2
# speedup vs baseline: 836.6317x; 836.6317x over previous
"""Additive (Bahdanau) attention log-softmax weights on 8 TRN2 NeuronCores.

Math (per batch b, head 0):
    qp = Q @ Wq^T ; kp = K @ Wk^T          (Wc = [Wq | Wk], both [D, D])
    logit[q, k] = Wl . tanh(qp[q] + kp[k] + bc) + bl + where(mask[k]==0, -1e9, 1.0)
    out[q, :]   = log_softmax(logit[q, :])

Distribution: pure data parallel, core c <- (batch b = c//2, q-half c%2),
no collectives.  Sparse-attention trick: keys with mask==0 only need
out = -1e9 - LSE (error O(1) vs magnitude 1e9), so the device only computes
tanh over the ~136 valid keys (host compacts + pads to V).

Device layout per core (q = 128 local rows, V padded valid keys, e = D = 512):
  - PE: qp^T/kp^T projections ([e,q],[e,kc]) with d on partitions.
  - DVE: arg[e, (q,kc)] = kp^T[e,kc] + (qp^T+bc)[e,q] via per-q tensor_scalar
    (bf16 in/out -> 4x mode).
  - ACT: tanh on [128, G*V] tiles (big free dim amortizes instr overhead).
  - PE: Wl-reduce per q-pair (N=2V) into PSUM partition bases {0,64}; the
    stationary is [128,64] with Wl in col 0 / zeros elsewhere so every PSUM
    partition is written (avoids uninitialized-read races).
  - copy PSUM->SBUF staging (engine-any), one gpsimd-queue DMA row-gather per
    4 tiles into a dense [64, 2V] logits tile, then log-softmax (exp is safe
    without max subtraction: |logit| <= sum|Wl| + 1 < 9).
"""

import numpy as np
import ml_dtypes
from contextlib import ExitStack

import concourse.bass as bass
import concourse.tile as tile
from concourse import bacc, mybir
from concourse.bass_utils import run_bass_kernel_spmd

F32 = mybir.dt.float32
BF16 = mybir.dt.bfloat16
AF = mybir.ActivationFunctionType

B, H, Lq, Lkv, D = 4, 1, 256, 256, 512
NCORES = 8
LQL = Lq // 2          # q rows per core
G = 32                 # q rows per tanh tile
NEG = -1.0e9

_nc_cache: dict[int, object] = {}


def _build(V: int, repeats: int = 1):
    """Build + schedule the per-core Bass graph for padded-valid-count V.

    repeats > 1 emits the full body that many times (for slope timing);
    the real kernel uses repeats=1 and the single "out" parameter."""
    W = 2 * V
    nc = bacc.Bacc(None, target_bir_lowering=False)

    p_qt = nc.declare_dram_parameter("qt", [128, 512], BF16, isOutput=False)
    p_kt = nc.declare_dram_parameter("kt", [128, 4 * V], BF16, isOutput=False)
    p_wct = nc.declare_dram_parameter("wct", [4, 128, 1024], BF16, isOutput=False)
    p_bcp = nc.declare_dram_parameter("bcp", [128, 4], F32, isOutput=False)
    p_wlg = nc.declare_dram_parameter("wlg", [128, 256], BF16, isOutput=False)
    p_bv = nc.declare_dram_parameter("bv", [64, W], F32, isOutput=False)
    # A single DRAM output shared by every repeat: the axon tunnel charges
    # a large fixed cost PER OUTPUT TENSOR (~80ms each, independent of size
    # or compute), so the repeat-slope timing NEFF must not scale its output
    # count with R.  Repeats overwrite the same region; the WAW is queue-
    # ordered and does not stall compute.
    p_out = nc.declare_dram_parameter("out", [64, W + 2], F32, isOutput=True)
    p_outs = [p_out] * repeats

    with ExitStack() as ctx:
        tc = ctx.enter_context(tile.TileContext(nc))
        const = ctx.enter_context(tc.tile_pool(name="const", bufs=1))
        apool = ctx.enter_context(tc.tile_pool(name="apool", bufs=6))
        tpool = ctx.enter_context(tc.tile_pool(name="tpool", bufs=9))
        spool = ctx.enter_context(tc.tile_pool(name="spool", bufs=3))
        psum = ctx.enter_context(tc.tile_pool(name="psum", bufs=8, space="PSUM"))

        for rep in range(repeats):
            # ---- loads (few big DMAs: each dma_start costs ~0.5us of queue
            #      dispatch, so tiles are packed column-wise into single tensors) --
            wct_t = []
            for ec in range(4):   # col = half*512 + dc*128 + e'
                t = const.tile([128, 1024], BF16, tag=f"wct{ec}", name=f"wct{ec}_r{rep}")
                wct_t.append(t)
            nc.sync.dma_start(wct_t[0][:], p_wct[0])
            bcp_t = const.tile([128, 4], F32, tag="bcp", name=f"bcp_r{rep}")
            nc.sync.dma_start(bcp_t[:], p_bcp[:])
            kt_t = const.tile([128, 4 * V], BF16, tag="kt", name=f"kt_r{rep}")   # col = dc*V + kc
            nc.sync.dma_start(kt_t[:], p_kt[:])
            qt_t = const.tile([128, 512], BF16, tag="qt", name=f"qt_r{rep}")     # col = dc*128 + q
            nc.sync.dma_start(qt_t[:], p_qt[:])
            for ec in range(1, 4):
                nc.sync.dma_start(wct_t[ec][:], p_wct[ec])
            wlg_t = const.tile([128, 256], BF16, tag="wlg", name=f"wlg_r{rep}")
            nc.sync.dma_start(wlg_t[:], p_wlg[:])
            bv_t = const.tile([64, W], F32, tag="bv", name=f"bv_r{rep}")
            nc.sync.dma_start(bv_t[:], p_bv[:])

            # ---- phase 2: tanh + Wl-reduce (phase-1 projections are emitted
            #      inside the first group, per ec, so the DVE stream interleaves
            #      them with the first arg-adds instead of blocking on wct3) ----
            qpbc = const.tile([128, D], F32, tag="qpbc", name=f"qpbc_r{rep}")     # col = ec*128 + q
            kpb = const.tile([128, 4 * V], BF16, tag="kpb", name=f"kpb_r{rep}")  # col = ec*V + kc
            lg = const.tile([64, W], F32, tag="lg", name=f"lg_r{rep}")  # row = q-pair, col = (q%2)*V+kc
            lgb = const.tile([64, W], F32, tag="lgb", name=f"lgb_r{rep}")
            ex = const.tile([64, W], F32, tag="ex", name=f"ex_r{rep}")
            sm = const.tile([64, 2], F32, tag="sm", name=f"sm_r{rep}")
            lsm = const.tile([64, 2], F32, tag="lsm", name=f"lsm_r{rep}")
            outv = const.tile([64, W], F32, tag="outv", name=f"outv_r{rep}")
            fill = const.tile([64, 2], F32, tag="fill", name=f"fill_r{rep}")
            # group sizes: small first group -> earlier ACT start; small last
            # group -> shorter serial tail
            GS = [16, 32, 32, 32, 16]
            qbase = 0
            sm_done = 0
            for qg, Gg in enumerate(GS):
                thts = []
                for ec in range(4):
                    if qg == 0:
                        ps2 = psum.tile([128, V], F32, tag="ps", name=f"psk{ec}_r{rep}")
                        for dc in range(4):
                            nc.tensor.matmul(
                                ps2[:],
                                wct_t[ec][:, 512 + dc * 128:512 + (dc + 1) * 128],
                                kt_t[:, dc * V:(dc + 1) * V],
                                start=(dc == 0), stop=(dc == 3))
                        nc.vector.tensor_copy(kpb[:, ec * V:(ec + 1) * V], ps2[:])
                        ps = psum.tile([128, 128], F32, tag="ps", name=f"psq{ec}_r{rep}")
                        for dc in range(4):
                            nc.tensor.matmul(
                                ps[:], wct_t[ec][:, dc * 128:(dc + 1) * 128],
                                qt_t[:, dc * 128:(dc + 1) * 128],
                                start=(dc == 0), stop=(dc == 3))
                        nc.vector.tensor_scalar_add(
                            qpbc[:, ec * 128:(ec + 1) * 128], ps[:],
                            bcp_t[:, ec:ec + 1])
                    a = apool.tile([128, Gg * V], BF16, tag="arg", name=f"arg{qg}_{ec}_r{rep}")
                    for j in range(Gg):
                        q = qbase + j
                        nc.vector.tensor_scalar_add(
                            a[:, j * V:(j + 1) * V],
                            kpb[:, ec * V:(ec + 1) * V],
                            qpbc[:, ec * 128 + q: ec * 128 + q + 1])
                    t = tpool.tile([128, Gg * V], BF16, tag="tht",
                                   name=f"tht{qg}_{ec}_r{rep}")
                    nc.scalar.activation(t[:], a[:], AF.Tanh)
                    thts.append(t)
                ntile = Gg // 4
                ptiles = [psum.tile([128, W], F32, tag="ps", name=f"pt{qg}_{i}_r{rep}")
                          for i in range(ntile)]
                # ec-outer keeps the PE stationary constant across a run of
                # matmuls (one ldweights per run on hardware)
                for ec, p_loc in [(ec, p) for ec in range(4)
                                  for p in range(Gg // 2)]:
                    tl, r = p_loc // 2, p_loc % 2
                    dst = ptiles[tl][64 * r: 64 * r + 64, :]
                    nc.tensor.matmul(
                        dst, wlg_t[:, ec * 64:ec * 64 + 64],
                        thts[ec][:, (2 * p_loc) * V:(2 * p_loc + 2) * V],
                        start=(ec == 0), stop=(ec == 3),
                        # the sim's zero-region tracker ignores the partition
                        # base, so the two disjoint 64-row groups per bank
                        # falsely collide; per-partition accumulation state is
                        # independent on HW
                        skip_group_check=True)
                for sg in range(ntile // 4):
                    st = spool.tile([128, 4 * W], F32, tag="st",
                                    name=f"st{qg}_{sg}_r{rep}")
                    for c in range(4):
                        nc.any.tensor_copy(st[:, c * W:(c + 1) * W],
                                           ptiles[sg * 4 + c][:])
                    # row r of the stage tile holds 4 pair-rows (one per column
                    # chunk) -> lg rows are filled in block order; the host
                    # unshard applies the matching permutation
                    for r in range(2):
                        base = qbase // 2 + sg * 8 + r * 4
                        nc.gpsimd.dma_start(
                            lg[base: base + 4, :], st[64 * r:64 * r + 1, :])
                qbase += Gg
                # softmax front per 32-row half (engine ops need 32-aligned
                # partition starts); exp's accum_out fuses the pair-sums
                while sm_done < qbase // 2 - 31:
                    r0 = sm_done
                    r1 = r0 + 32
                    nc.vector.tensor_add(lgb[r0:r1, :], lg[r0:r1, :], bv_t[r0:r1, :])
                    nc.scalar.activation(ex[r0:r1, 0:V], lgb[r0:r1, 0:V], AF.Exp,
                                         accum_out=sm[r0:r1, 0:1])
                    nc.scalar.activation(ex[r0:r1, V:W], lgb[r0:r1, V:W], AF.Exp,
                                         accum_out=sm[r0:r1, 1:2])
                    sm_done = r1

            # ---- softmax tail: Ln grouped once (one act-table switch) ----
            nc.scalar.activation(lsm[:], sm[:], AF.Ln)
            nc.vector.tensor_scalar_sub(outv[:, 0:V], lgb[:, 0:V], lsm[:, 0:1])
            nc.vector.tensor_scalar_sub(outv[:, V:W], lgb[:, V:W], lsm[:, 1:2])
            nc.vector.tensor_scalar(
                fill[:], lsm[:], -1.0, NEG,
                op0=mybir.AluOpType.mult, op1=mybir.AluOpType.add)
            nc.sync.dma_start(p_outs[rep][:, 0:W], outv[:])
            nc.scalar.dma_start(p_outs[rep][:, W:W + 2], fill[:])

    nc.compile()
    return nc


def _prep(queries, keys, values, mask, Wc, bc, Wl, bl):
    """Host-side sharding: returns (V, in_maps, idx_valid, idx_masked)."""
    mask = np.asarray(mask)
    idx_v = [np.nonzero(mask[b])[0] for b in range(B)]
    idx_m = [np.nonzero(mask[b] == 0)[0] for b in range(B)]
    maxv = max(len(ix) for ix in idx_v)
    V = max(136, -(-maxv // 8) * 8)

    bf = ml_dtypes.bfloat16
    wct_full = np.asarray(Wc, np.float32).T.astype(bf)       # [2D, D]
    wct = np.empty((4, 128, 1024), bf)
    for ec in range(4):
        for dc in range(4):
            wct[ec, :, dc * 128:(dc + 1) * 128] = \
                wct_full[dc * 128:(dc + 1) * 128, ec * 128:(ec + 1) * 128]
            wct[ec, :, 512 + dc * 128:512 + (dc + 1) * 128] = \
                wct_full[D + dc * 128:D + (dc + 1) * 128, ec * 128:(ec + 1) * 128]
    bcp = np.ascontiguousarray(np.asarray(bc, np.float32).reshape(4, 128).T)
    wlg = np.zeros((128, 256), bf)
    wlg[:, 0::64] = np.asarray(Wl, np.float32)[0].reshape(4, 128).T.astype(bf)
    blv = float(np.asarray(bl, np.float32)[0])

    q_np = np.asarray(queries, np.float32)
    k_np = np.asarray(keys, np.float32)
    in_maps = []
    for c in range(NCORES):
        b, qh = c // 2, c % 2
        qt_d = q_np[b, 0, qh * LQL:(qh + 1) * LQL, :].T.astype(bf)   # [D, LQL]
        qt = np.ascontiguousarray(
            qt_d.reshape(4, 128, LQL).transpose(1, 0, 2).reshape(128, 512))
        ktc = np.zeros((D, V), bf)
        ktc[:, :len(idx_v[b])] = k_np[b, 0, idx_v[b], :].T.astype(bf)
        kt = np.ascontiguousarray(
            ktc.reshape(4, 128, V).transpose(1, 0, 2).reshape(128, 4 * V))
        bvrow = np.full(V, NEG, np.float32)
        bvrow[:len(idx_v[b])] = 1.0 + blv
        bv = np.tile(np.concatenate([bvrow, bvrow]), (64, 1))
        in_maps.append({
            "qt": qt, "kt": kt, "wct": wct,
            "bcp": bcp, "wlg": wlg, "bv": np.ascontiguousarray(bv),
        })
    return V, in_maps, idx_v, idx_m


def _unshard_rows(o, V):
    """[64, 2V+2] device rows -> ([pair, q%2, kc] values, [pair, q%2] fill).

    lg row (qg, sg, r, c) holds q-pair qg*16 + sg*8 + 2*c + r."""
    W = 2 * V
    perm = np.empty(64, np.int64)
    for qg in range(4):
        for sg in range(2):
            for r in range(2):
                for cc in range(4):
                    perm[qg * 16 + sg * 8 + 2 * cc + r] = \
                        qg * 16 + sg * 8 + r * 4 + cc
    return o[perm, :W].reshape(64, 2, V), o[perm, W:W + 2]


def kernel(queries, keys, values, mask, Wc, bc, Wl, bl):
    V, in_maps, idx_v, idx_m = _prep(queries, keys, values, mask, Wc, bc, Wl, bl)
    if V not in _nc_cache:
        _nc_cache[V] = _build(V)
    nc = _nc_cache[V]
    res = run_bass_kernel_spmd(nc, in_maps, core_ids=list(range(NCORES))).results

    full = np.empty((B, Lq, Lkv), np.float32)
    for c in range(NCORES):
        b, qh = c // 2, c % 2
        vals, fl = _unshard_rows(np.asarray(res[c]["out"], np.float32), V)
        nv = len(idx_v[b])
        blk = full[b, qh * LQL:(qh + 1) * LQL]          # [128, Lkv]
        blk[:, idx_v[b]] = vals[:, :, :nv].reshape(LQL, nv)
        blk[:, idx_m[b]] = fl.reshape(LQL, 1)
    return full

